# revision 48
# baseline (speedup 1.0000x reference)
"""Self-contained Trainium2 Bass kernel for a 1-layer transformer encoder.

Model (fp32 reference):
  x = (emb[input_seq] + pos) * sqrt(D)
  k = x@wk+bk ; q = x@wq+bq ; v = x@wv+bv
  scores[b,i,j] = sum_d k[b,i,d]*q[b,j,d] / sqrt(D)
  attn = softmax(scores, axis=-1) @ v
  r = LN(x + attn) ; ff = gelu(r@w1+b1)@w2+b2 ; out = LN(r + ff)

Sharding: 8 cores; core c handles batch c//2, sequence-half c%2.  Each core
receives its batch's full sequence rolled by -1024*h so its half is local
rows 0..1023 (softmax over keys is permutation-invariant, so one SPMD
program serves both halves).  QKV for the full local sequence is computed
on-core (duplicated across the pair); no collectives.

Precision: matmuls run single-pass float32r (full-rate fp32, ~9 mantissa
bits) using a fused matrix M = wk @ (wq/sqrt(D)).T (host-precomputed; the
per-row bias terms cancel in softmax except the bk term which rides in as
a rank-1 t2 correction when bk != 0):
  u = x@M ; scores = u @ x^T (+ 1*t2)
Softmax probabilities, v, the embedding table, w2 and the gelu activations
are bf16; the attention denominator comes from the exp accumulator in f32.
Measured end-to-end rel err ~4.4e-3 vs the 2e-2 gate.

Schedule notes (tlsim-guided):
- Phase 2 is software-pipelined: per step, pT transposes+copies(i-1) are
  emitted first (DVE copies lead the DVE queue so ps_t recycling never
  gates PE), then scores(i), exps(i), attn(i-1), denominator(i).
- LN1 is deferred and batched (rows 0..5 early, 6..7 on DVE at the end):
  one Sqrt act-table load instead of per-i Ln/Exp table thrash (1283ns
  per load, and exp/ln/sqrt live in different table sets).
- scores(0)+softmax(0) are prefilled at the end of phase 1 (jb0/jb1 on
  the then-idle attn-tag PSUM banks) so the first softmax hides under
  the phase-boundary drain.
- idx DMA is first on the sync queue (it gates all embedding gathers;
  per-queue DMA completion waits round up to 8-transfer generations, so
  queue order and crowding matter).  FFN weight DMAs are WAW-gated via
  1-element marker writes so they cannot front-run the gathers on the
  shared DMA fabric (queues run ahead of real time, so emission position
  alone does not delay a dependency-free DMA).
"""

import math

import ml_dtypes
import numpy as np

_B, _S, _D, _DFF, _V = 4, 2048, 512, 2048, 50257
_P = 128
_NCORES = 8
_SQRT_D = math.sqrt(_D)
_EPS = 1e-5

_NT = _S // _P          # 16 sequence tiles
_NI = (_S // 2) // _P   # 8 row tiles per core half
_KC = _D // _P          # 4 contraction chunks over D
_FC = _DFF // _P        # 16 contraction chunks over DFF
_JB = _S // 512         # 4 key blocks of 512

_CACHE = {}


def _pos_table():
    # Mirrors reference pos_embedding in float32.
    pos = np.arange(_S, dtype=np.float32)[:, None]
    i = np.arange(_D, dtype=np.float32)[None, :]
    ang = pos / np.power(np.float32(10000.0), np.float32(2.0) * i / np.float32(_D))
    even = (np.arange(_D) % 2 == 0)[None, :]
    return np.where(even, np.sin(ang), np.cos(ang)).astype(np.float32)


def _round_f32r(a):
    # float32r keeps the top 9 mantissa bits; round-to-nearest on the low 14.
    b = np.ascontiguousarray(a, dtype=np.float32).view(np.uint32)
    b = (b + np.uint32(0x2000)) & np.uint32(0xFFFFC000)
    return b.view(np.float32)


def _build_nc(zero_bk=False, zero_bv=False, zero_b2=False, unit_g=False,
              zero_lb=False):
    import concourse.bass as bass
    import concourse.mybir as mybir
    import concourse.tile as tile
    from concourse import bacc
    from concourse.masks import make_identity

    f32 = mybir.dt.float32
    f32r = mybir.dt.float32r
    bf16 = mybir.dt.bfloat16
    i32 = mybir.dt.int32
    i16 = mybir.dt.int16
    AF = mybir.ActivationFunctionType
    OP = mybir.AluOpType
    AX = mybir.AxisListType.X

    nc = bacc.Bacc("TRN2", target_bir_lowering=False, debug=False,
                   num_devices=_NCORES)

    idx_d = nc.dram_tensor("idx", [_P, _NT], i32, kind="ExternalInput")
    # Compact per-core embedding table: host gathers the <=S unique rows this
    # core's batch touches (device still performs the data-dependent gather).
    emb_d = nc.dram_tensor("emb", [_S, _D], bf16, kind="ExternalInput")
    pos_d = nc.dram_tensor("pos", [_S, _D], f32, kind="ExternalInput")
    mhi_d = nc.dram_tensor("m_hi", [_D, _D], f32r, kind="ExternalInput")
    wv_d = nc.dram_tensor("wv", [_D, _D], f32r, kind="ExternalInput")
    w1_d = nc.dram_tensor("w1", [_D, _DFF], f32r, kind="ExternalInput")
    w2_d = nc.dram_tensor("w2", [_DFF, _D], bf16, kind="ExternalInput")
    c2c_d = nc.dram_tensor("c2c", [_P, _KC], f32r, kind="ExternalInput")
    bvb_d = nc.dram_tensor("bvb", [_P, _D], f32, kind="ExternalInput")
    b1c_d = nc.dram_tensor("b1c", [_P, _FC], f32, kind="ExternalInput")
    b2b_d = nc.dram_tensor("b2b", [_P, _D], f32, kind="ExternalInput")
    gb_d = nc.dram_tensor("gb", [_P, _D], f32, kind="ExternalInput")
    lbb_d = nc.dram_tensor("lbb", [_P, _D], f32, kind="ExternalInput")
    out_d = nc.dram_tensor("out", [_S // 2, _D], f32, kind="ExternalOutput")

    with tile.TileContext(nc) as tc:
        consts = tc.alloc_tile_pool(name="consts", bufs=1)
        id_f = consts.tile([_P, _P], f32, name="id_f")
        make_identity(nc, id_f[:])
        id_r = consts.tile([_P, _P], f32r, name="id_r")
        nc.vector.tensor_copy(out=id_r[:], in_=id_f[:])
        id_b = consts.tile([_P, _P], bf16, name="id_b")
        nc.vector.tensor_copy(out=id_b[:], in_=id_f[:])
        ones_r = consts.tile([1, _P], f32, name="ones_f")
        nc.vector.memset(ones_r[:], 1.0)
        ones_rr = consts.tile([1, _P], f32r, name="ones_rr")
        nc.vector.tensor_copy(out=ones_rr[:], in_=ones_r[:])
        eps_t = consts.tile([_P, 1], f32, name="eps_t")
        nc.vector.memset(eps_t[:], _EPS)
        c2c = bvb = b2b = gb = lbb = None
        if not zero_bk:
            c2c = consts.tile([_P, _KC], f32r, name="c2c")
            nc.scalar.dma_start(out=c2c[:], in_=c2c_d[:, :])
        if not zero_bv:
            bvb = consts.tile([_P, _D], f32, name="bvb")
            nc.scalar.dma_start(out=bvb[:], in_=bvb_d[:, :])
        b1c = consts.tile([_P, _FC], f32, name="b1c")
        if not zero_b2:
            b2b = consts.tile([_P, _D], f32, name="b2b")
            nc.scalar.dma_start(out=b2b[:], in_=b2b_d[:, :])
        if not (unit_g and zero_lb):
            gb = consts.tile([_P, _D], f32, name="gb")
            nc.scalar.dma_start(out=gb[:], in_=gb_d[:, :])
            lbb = consts.tile([_P, _D], f32, name="lbb")
            nc.scalar.dma_start(out=lbb[:], in_=lbb_d[:, :])

        xhalf = tc.alloc_tile_pool(name="xhalf", bufs=1)
        x_sb = xhalf.tile([_P, _NI, _D], f32r, name="x_sb")

        acts = tc.alloc_tile_pool(name="acts", bufs=1)
        xT = acts.tile([_P, _KC, _S], f32r, name="xT")
        uT = acts.tile([_P, _KC, _S // 2], f32r, name="uT")
        v_sb = acts.tile([_P, _NT, _D], bf16, name="v_sb")
        t2_sb = None if zero_bk else acts.tile([1, _S], f32r, name="t2_sb")

        # ---------------- Phase 1: embed, transpose, u = x@M, v ----------
        # idx FIRST on the sync queue: it gates every embedding gather, and
        # the shared DMA fabric serves transfers in issue order.
        p1t = tc.alloc_tile_pool(name="p1t", bufs=1)
        idx_sb = p1t.tile([_P, _NT], i32, name="idx_sb")
        nc.sync.dma_start(out=idx_sb[:], in_=idx_d[:, :])
        nc.sync.dma_start(out=b1c[:], in_=b1c_d[:, :])

        p1 = tc.alloc_tile_pool(name="p1", bufs=1)
        wv_sb = p1.tile([_P, _KC, _D], f32r, name="wv_sb")
        mhi_sb = p1.tile([_P, _KC, _D], f32r, name="mhi_sb")

        for c in range(_KC):
            nc.sync.dma_start(out=wv_sb[:, c, :], in_=wv_d[c * _P:(c + 1) * _P, :])

        psp = tc.alloc_tile_pool(name="psp", bufs=1, space="PSUM")

        def emit_u(ibl):
            rsl = slice(ibl * 512, (ibl + 1) * 512)
            for oc in range(_KC):
                ps_u = psp.tile([_P, 512], f32, name="ps_u", tag="mm", bufs=4)
                for c in range(_KC):
                    nc.tensor.matmul(out=ps_u[:],
                                     lhsT=mhi_sb[:, c, oc * _P:(oc + 1) * _P],
                                     rhs=xT[:, c, rsl],
                                     start=(c == 0), stop=(c == _KC - 1))
                nc.scalar.activation(out=uT[:, oc, rsl], in_=ps_u[:],
                                     func=AF.Identity, scale=1.0)

        def emit_v(t):
            ps_v = psp.tile([_P, 512], f32, name="ps_v", tag="mm", bufs=4)
            for c in range(_KC):
                nc.tensor.matmul(out=ps_v[:],
                                 lhsT=xT[:, c, t * _P:(t + 1) * _P],
                                 rhs=wv_sb[:, c, :],
                                 start=(c == 0), stop=(c == _KC - 1))
            if zero_bv:
                # t>=14 on ACT: keeps the DVE queue clear for the prefill maxes.
                if t % 2 == 0 and t < 14:
                    nc.vector.tensor_copy(out=v_sb[:, t, :], in_=ps_v[:])
                else:
                    nc.scalar.activation(out=v_sb[:, t, :], in_=ps_v[:],
                                         func=AF.Identity, scale=1.0)
            else:
                nc.vector.tensor_tensor(out=v_sb[:, t, :], in0=ps_v[:], in1=bvb[:],
                                        op=OP.add)

        pf_m4 = acts.tile([_P, _JB], f32, name="pf_m4")
        pf_ps = [None] * _JB

        def emit_pf(jb, tag, bufs):
            ps_sj = psp.tile([_P, 512], f32, name="ps_s", tag=tag, bufs=bufs)
            pf_ps[jb] = ps_sj
            jsl = slice(jb * 512, (jb + 1) * 512)
            for c in range(_KC):
                nc.tensor.matmul(out=ps_sj[:],
                                 lhsT=uT[:, c, 0:_P], rhs=xT[:, c, jsl],
                                 start=(c == 0), stop=(zero_bk and c == _KC - 1))
            if not zero_bk:
                nc.tensor.matmul(out=ps_sj[:], lhsT=ones_rr[0:1, :],
                                 rhs=t2_sb[0:1, jsl], start=False, stop=True)
            nc.vector.reduce_max(out=pf_m4[:, jb:jb + 1], in_=ps_sj[:], axis=AX)

        def emit_t2(jb):
            ps_m = psp.tile([_P, 512], f32, name="ps_m", tag="mm", bufs=4)
            jsl = slice(jb * 512, (jb + 1) * 512)
            for c in range(_KC):
                nc.tensor.matmul(out=ps_m[0:1, :], lhsT=c2c[:, c:c + 1],
                                 rhs=xT[:, c, jsl],
                                 start=(c == 0), stop=(c == _KC - 1))
            nc.vector.tensor_copy(out=t2_sb[0:1, jsl], in_=ps_m[0:1, :])

        for t in range(_NT):
            if t < _KC:
                # mhi staggered behind pos tiles on the ACT queue; first
                # consumed by emit_u(0) at t == 4, after these land.
                nc.scalar.dma_start(out=mhi_sb[:, t, :],
                                    in_=mhi_d[t * _P:(t + 1) * _P, :])
            xg = p1t.tile([_P, _D], bf16, name="xg", tag="xg", bufs=3)
            nc.gpsimd.indirect_dma_start(
                out=xg[:], out_offset=None, in_=emb_d[:, :],
                in_offset=bass.IndirectOffsetOnAxis(ap=idx_sb[:, t:t + 1], axis=0))
            pos_t = p1t.tile([_P, _D], f32, name="pos_t", tag="pos_t", bufs=3)
            nc.scalar.dma_start(out=pos_t[:], in_=pos_d[t * _P:(t + 1) * _P, :])
            if t < _NI:
                x_f = x_sb[:, t, :]
            else:
                x_f = p1t.tile([_P, _D], f32r, name="x_f", tag="x_f", bufs=3)[:]
            nc.vector.tensor_tensor(out=x_f, in0=xg[:], in1=pos_t[:], op=OP.add)
            ps_x = psp.tile([_P, _KC, _P], f32r, name="ps_x", tag="tp", bufs=2)
            for c in range(_KC):
                nc.tensor.transpose(out=ps_x[:, c, :], in_=x_f[:, c * _P:(c + 1) * _P],
                                    identity=id_r[:])
            sl = slice(t * _P, (t + 1) * _P)
            nc.scalar.activation(out=xT[:, :, sl], in_=ps_x[:, :, :],
                                 func=AF.Identity, scale=1.0)
            # v lags one tile so the ACT xT copy has a full tile period to
            # land before the v matmuls consume it.
            if t > 0:
                emit_v(t - 1)
            if t == 4:
                emit_u(0)
            if t == 8:
                emit_u(1)
            if t in (10, 12) and zero_bk:
                # prefill scores jb0/jb1 on the phase-1-idle attn-tag banks
                # (keys 0..1023 are transposed by t == 8)
                emit_pf(t // 2 - 5, "attn", 2)
        if not zero_bk:
            for jb in range(_JB):
                emit_t2(jb)

        # Prefill scores for jb2/jb3 (jb0/jb1 were emitted mid-loop on the
        # then-idle attn-tag banks); softmax(0) latency hides under v(15),
        # the phase-boundary drain, and scores(1).
        for jb in ((2, 3) if zero_bk else (0, 1, 2, 3)):
            emit_pf(jb, "mm", 4)
        pf_mneg = acts.tile([_P, 1], f32, name="pf_mneg")
        nc.vector.reduce_max(out=pf_mneg[:], in_=pf_m4[:, :], axis=AX, negate=True)
        emit_v(_NT - 1)
        pf_p = acts.tile([_P, _S], bf16, name="pf_p")
        pf_s4 = acts.tile([_P, _JB], f32, name="pf_s4")
        for jb in range(_JB):
            nc.scalar.activation(out=pf_p[:, jb * 512:(jb + 1) * 512],
                                 in_=pf_ps[jb][:], func=AF.Exp,
                                 bias=pf_mneg[:, 0:1], scale=1.0,
                                 accum_out=pf_s4[:, jb:jb + 1])
        pf_ssum = acts.tile([_P, 1], f32, name="pf_ssum")
        nc.vector.reduce_sum(out=pf_ssum[:], in_=pf_s4[:, :], axis=AX)
        pf_rinv = acts.tile([_P, 1], f32, name="pf_rinv")
        nc.vector.reciprocal(out=pf_rinv[:], in_=pf_ssum[:])

        p1.release()
        p1t.release()

        # ---------------- Phase 2: attention (pipelined) + batched LN1 ----
        rpool = tc.alloc_tile_pool(name="rpool", bufs=1, side="right")
        r_sb = rpool.tile([_P, _NI, _D], f32r, name="r_sb")
        mu_all = rpool.tile([_P, _NI], f32, name="mu_all")
        var_all = rpool.tile([_P, _NI], f32, name="var_all")
        std_all = rpool.tile([_P, _NI], f32, name="std_all")
        rstd_all = rpool.tile([_P, _NI], f32, name="rstd_all")
        # FFN weights: tiles here, DMAs issued mid-phase-2 from the scalar
        # queue so the transfers ride the then-idle HWDGE path instead of
        # clogging the shared DMA fabric in front of the embedding gathers.
        w1a = rpool.tile([_P, _KC, _DFF // 2], f32r, name="w1a")
        w1b = rpool.tile([_P, _KC, _DFF // 2], f32r, name="w1b")
        w2a = rpool.tile([_P, _FC // 2, _D], bf16, name="w2a")
        w2b = rpool.tile([_P, _FC // 2, _D], bf16, name="w2b")
        rT = rpool.tile([_P, _KC, _S // 2], f32r, name="rT")

        zpool = tc.alloc_tile_pool(name="zpool", bufs=1)
        z_sb = zpool.tile([_P, _NI, _D], f32, name="z_sb")

        p2 = tc.alloc_tile_pool(name="p2", bufs=1)

        def emit_ln1_batch(lo, hi, aeng=None):
            # One Sqrt over the batched variances (table loaded once), then
            # DVE reciprocal + per-row normalize.
            nc.scalar.activation(out=std_all[:, lo:hi], in_=var_all[:, lo:hi],
                                 func=AF.Sqrt, bias=eps_t[:, 0:1], scale=1.0)
            nc.vector.reciprocal(out=rstd_all[:, lo:hi], in_=std_all[:, lo:hi])
            for i in range(lo, hi):
                if unit_g and zero_lb:
                    (aeng or nc.gpsimd).tensor_scalar(out=r_sb[:, i, :], in0=z_sb[:, i, :],
                                            scalar1=mu_all[:, i:i + 1],
                                            scalar2=rstd_all[:, i:i + 1],
                                            op0=OP.subtract, op1=OP.mult)
                else:
                    t1 = p2.tile([_P, _D], f32, name="t1", tag="t1", bufs=2)
                    nc.vector.tensor_scalar(out=t1[:], in0=z_sb[:, i, :],
                                            scalar1=mu_all[:, i:i + 1],
                                            scalar2=rstd_all[:, i:i + 1],
                                            op0=OP.subtract, op1=OP.mult)
                    t2t = p2.tile([_P, _D], f32, name="t2t", tag="t2t", bufs=2)
                    nc.gpsimd.tensor_tensor(out=t2t[:], in0=t1[:], in1=gb[:],
                                            op=OP.mult)
                    nc.gpsimd.tensor_tensor(out=r_sb[:, i, :], in0=t2t[:], in1=lbb[:],
                                            op=OP.add)

        pending = (0, pf_p, pf_rinv)
        for step in range(1, _NI + 1):
            # -- pT transposes + copies for step-1 FIRST: the DVE copies lead
            #    this period's DVE queue so the ps_t recycling never gates PE --
            if pending is not None:
                (i0, p_prev, rinv_prev) = pending
                pT = p2.tile([_P, _NT, _P], bf16, name="pT", tag="pT", bufs=2)
                for g in range(4):
                    ps_t = psp.tile([_P, 4, _P], bf16, name="ps_t", tag="tp", bufs=2)
                    for q in range(4):
                        jt = 4 * g + q
                        nc.tensor.transpose(out=ps_t[:, q, :],
                                            in_=p_prev[:, jt * _P:(jt + 1) * _P],
                                            identity=id_b[:])
                    nc.vector.tensor_copy(out=pT[:, 4 * g:4 * (g + 1), :],
                                          in_=ps_t[:, :, :])

            # -- scores(step) matmuls + row maxes --------------------------
            if step < _NI:
                i = step
                isl = slice(i * _P, (i + 1) * _P)
                ps_s = []
                m4 = p2.tile([_P, _JB], f32, name="m4", tag="m4", bufs=2)
                for jb in range(_JB):
                    ps_sj = psp.tile([_P, 512], f32, name="ps_s", tag="mm", bufs=4)
                    ps_s.append(ps_sj)
                    jsl = slice(jb * 512, (jb + 1) * 512)
                    for c in range(_KC):
                        nc.tensor.matmul(out=ps_sj[:],
                                         lhsT=uT[:, c, isl], rhs=xT[:, c, jsl],
                                         start=(c == 0),
                                         stop=(zero_bk and c == _KC - 1))
                    if not zero_bk:
                        nc.tensor.matmul(out=ps_sj[:], lhsT=ones_rr[0:1, :],
                                         rhs=t2_sb[0:1, jsl], start=False, stop=True)
                    nc.vector.reduce_max(out=m4[:, jb:jb + 1], in_=ps_sj[:], axis=AX)
                mneg = p2.tile([_P, 1], f32, name="mneg", tag="mneg", bufs=2)
                nc.vector.reduce_max(out=mneg[:], in_=m4[:, :], axis=AX, negate=True)

            # -- exps for this step ---------------------------------------
            if step < _NI:
                p_cur = p2.tile([_P, _S], bf16, name="p_sb", tag="p_sb", bufs=2)
                s4 = p2.tile([_P, _JB], f32, name="s4", tag="s4", bufs=2)
                for jb in range(_JB):
                    nc.scalar.activation(out=p_cur[:, jb * 512:(jb + 1) * 512],
                                         in_=ps_s[jb][:], func=AF.Exp,
                                         bias=mneg[:, 0:1], scale=1.0,
                                         accum_out=s4[:, jb:jb + 1])

            # -- early LN1 batch: rows 0..5 have stats by now; the Sqrt's
            #    one-time table load and the Pool applies overlap the rest of
            #    the i-loop so r is ready the moment phase 3 starts ----------
            if step == _NI - 1:
                emit_ln1_batch(0, _NI - 2)

            # -- attention matmuls + residual/stats for step-1 ------------
            if pending is not None:
                ps_a = psp.tile([_P, _D], f32, name="ps_a", tag="attn", bufs=2)
                for jt in range(_NT):
                    nc.tensor.matmul(out=ps_a[:], lhsT=pT[:, jt, :],
                                     rhs=v_sb[:, jt, :],
                                     start=(jt == 0), stop=(jt == _NT - 1))
                zt = p2.tile([_P, _D], f32, name="zt", tag="zt", bufs=2)
                nc.scalar.activation(out=zt[:], in_=ps_a[:], func=AF.Identity,
                                     scale=rinv_prev[:, 0:1])
                zeng = nc.vector if i0 >= _NI - 2 else nc.gpsimd
                zeng.tensor_tensor(out=z_sb[:, i0, :], in0=zt[:],
                                   in1=x_sb[:, i0, :], op=OP.add)
                stats = p2.tile([_P, 6], f32, name="stats", tag="stats", bufs=2)
                nc.vector.bn_stats(out=stats[:], in_=z_sb[:, i0, :])
                mv = p2.tile([_P, 2], f32, name="mv", tag="mv", bufs=2)
                nc.vector.bn_aggr(out=mv[:], in_=stats[:])
                nc.vector.tensor_copy(out=mu_all[:, i0:i0 + 1], in_=mv[:, 0:1])
                nc.vector.tensor_copy(out=var_all[:, i0:i0 + 1], in_=mv[:, 1:2])

            # -- denominator for this step --------------------------------
            if step < _NI:
                ssum = p2.tile([_P, 1], f32, name="ssum", tag="ssum", bufs=2)
                nc.vector.reduce_sum(out=ssum[:], in_=s4[:, :], axis=AX)
                rinv = p2.tile([_P, 1], f32, name="rinv", tag="rinv", bufs=2)
                nc.vector.reciprocal(out=rinv[:], in_=ssum[:])
                pending = (i, p_cur, rinv)
            else:
                pending = None

            # FFN weight prefetch: WAW-gate each DMA on this step's rinv (a
            # 1-element marker write) so the transfer cannot front-run the
            # phase-1 embedding gathers on the shared DMA engines.
            def gated_wdma(wt, src_ap):
                nc.gpsimd.tensor_copy(out=wt[0:1, 0:1, 0:1],
                                      in_=pending[2][0:1, 0:1])
                nc.scalar.dma_start(out=wt[:], in_=src_ap)
            if step == 1:
                gated_wdma(w1a, w1_d[:, 0:_DFF // 2].rearrange("(c p) n -> p c n", p=_P))
            elif step == 2:
                gated_wdma(w1b, w1_d[:, _DFF // 2:].rearrange("(c p) n -> p c n", p=_P))
            elif step == 4:
                gated_wdma(w2a, w2_d[0:_DFF // 2, :].rearrange("(c p) n -> p c n", p=_P))
            elif step == 6:
                gated_wdma(w2b, w2_d[_DFF // 2:, :].rearrange("(c p) n -> p c n", p=_P))
        # final two row tiles (applies on DVE: Pool is mid-drain by now)
        emit_ln1_batch(_NI - 2, _NI, aeng=nc.vector)

        p2.release()
        zpool.release()
        acts.release()
        xhalf.release()

        # ---------------- Phase 3: FFN + LN2 ----------------
        p3 = tc.alloc_tile_pool(name="p3", bufs=1)

        def emit_rt(i):
            ps_rt = psp.tile([_P, _KC, _P], f32r, name="ps_rt", tag="tp", bufs=2)
            for c in range(_KC):
                nc.tensor.transpose(out=ps_rt[:, c, :],
                                    in_=r_sb[:, i, c * _P:(c + 1) * _P],
                                    identity=id_r[:])
            nc.vector.tensor_copy(out=rT[:, :, i * _P:(i + 1) * _P], in_=ps_rt[:, :, :])

        for i in range(4):
            emit_rt(i)
        gT0 = p3.tile([_P, _FC, 512], bf16, name="gT0")
        gT1 = p3.tile([_P, _FC, 512], bf16, name="gT1")
        for ib, gT in ((0, gT0), (1, gT1)):
            if ib == 1:
                for i in range(4, _NI):
                    emit_rt(i)
            for fc in range(_FC):
                ps_h = psp.tile([_P, 512], f32, name="ps_h", tag="mm", bufs=4)
                w1h = w1a if fc < _FC // 2 else w1b
                fcl = fc if fc < _FC // 2 else fc - _FC // 2
                for c in range(_KC):
                    nc.tensor.matmul(out=ps_h[:],
                                     lhsT=w1h[:, c, fcl * _P:(fcl + 1) * _P],
                                     rhs=rT[:, c, ib * 512:(ib + 1) * 512],
                                     start=(c == 0), stop=(c == _KC - 1))
                nc.scalar.activation(out=gT[:, fc, :], in_=ps_h[:], func=AF.Gelu,
                                     bias=b1c[:, fc:fc + 1], scale=1.0)
        for i in range(_NI):
            ib, il = divmod(i, 4)
            gT = gT0 if ib == 0 else gT1
            ps_o = psp.tile([_P, _D], f32, name="ps_o", tag="attn", bufs=2)
            for fc in range(_FC):
                w2h = w2a if fc < _FC // 2 else w2b
                fcl = fc if fc < _FC // 2 else fc - _FC // 2
                nc.tensor.matmul(out=ps_o[:],
                                 lhsT=gT[:, fc, il * _P:(il + 1) * _P],
                                 rhs=w2h[:, fcl, :],
                                 start=(fc == 0), stop=(fc == _FC - 1))
            t3 = p3.tile([_P, _D], f32, name="t3", tag="t3", bufs=2)
            nc.vector.tensor_tensor(out=t3[:], in0=ps_o[:], in1=r_sb[:, i, :],
                                    op=OP.add)
            if zero_b2:
                z2 = t3
            else:
                z2 = p3.tile([_P, _D], f32, name="z2", tag="z2", bufs=2)
                nc.gpsimd.tensor_tensor(out=z2[:], in0=t3[:], in1=b2b[:], op=OP.add)
            stats2 = p3.tile([_P, 6], f32, name="stats2", tag="stats2", bufs=2)
            nc.vector.bn_stats(out=stats2[:], in_=z2[:])
            mv2 = p3.tile([_P, 2], f32, name="mv2", tag="mv2", bufs=2)
            nc.vector.bn_aggr(out=mv2[:], in_=stats2[:])
            std2 = p3.tile([_P, 1], f32, name="std2", tag="std2", bufs=2)
            nc.scalar.activation(out=std2[:], in_=mv2[:, 1:2], func=AF.Sqrt,
                                 bias=eps_t[:, 0:1], scale=1.0)
            rstd2 = p3.tile([_P, 1], f32, name="rstd2", tag="rstd2", bufs=2)
            nc.vector.reciprocal(out=rstd2[:], in_=std2[:])
            out_t = p3.tile([_P, _D], f32, name="out_t", tag="out_t", bufs=3)
            if unit_g and zero_lb:
                nc.vector.tensor_scalar(out=out_t[:], in0=z2[:], scalar1=mv2[:, 0:1],
                                        scalar2=rstd2[:, 0:1],
                                        op0=OP.subtract, op1=OP.mult)
            else:
                t4 = p3.tile([_P, _D], f32, name="t4", tag="t4", bufs=2)
                nc.vector.tensor_scalar(out=t4[:], in0=z2[:], scalar1=mv2[:, 0:1],
                                        scalar2=rstd2[:, 0:1],
                                        op0=OP.subtract, op1=OP.mult)
                t5 = p3.tile([_P, _D], f32, name="t5", tag="t5", bufs=2)
                nc.gpsimd.tensor_tensor(out=t5[:], in0=t4[:], in1=gb[:], op=OP.mult)
                nc.gpsimd.tensor_tensor(out=out_t[:], in0=t5[:], in1=lbb[:], op=OP.add)
            nc.sync.dma_start(out=out_d[i * _P:(i + 1) * _P, :], in_=out_t[:])

        psp.release()
        p3.release()
        rpool.release()
        consts.release()

    nc.compile()
    return nc


def _get_nc(flags=(False, False, False, False, False)):
    if flags not in _CACHE:
        _CACHE[flags] = _build_nc(*flags)
    return _CACHE[flags]


def _make_in_maps(inp):
    f32 = np.float32
    emb_full = np.asarray(inp["emb"])
    pos_s = _pos_table() * f32(_SQRT_D)

    wk64 = np.asarray(inp["wk"], np.float64)
    wqp64 = np.asarray(inp["wq"], np.float64) / _SQRT_D
    m64 = wk64 @ wqp64.T
    m_hi = _round_f32r(m64.astype(np.float32))
    c2 = (wqp64 @ np.asarray(inp["bk"], np.float64)).astype(f32)

    def col(bias, nchunk):
        return np.ascontiguousarray(np.asarray(bias, f32).reshape(nchunk, _P).T)

    def bcast(bias):
        return np.ascontiguousarray(np.broadcast_to(np.asarray(bias, f32), (_P, _D)))

    shared = {
        "m_hi": np.ascontiguousarray(m_hi),
        "wv": np.ascontiguousarray(inp["wv"], dtype=f32),
        "w1": np.ascontiguousarray(inp["w1"], dtype=f32),
        "w2": np.ascontiguousarray(inp["w2"], dtype=f32).astype(ml_dtypes.bfloat16),
        "c2c": col(_round_f32r(c2), _KC),
        "bvb": bcast(inp["bv"]),
        "b1c": col(inp["b1"], _FC),
        "b2b": bcast(inp["b2"]),
        "gb": bcast(inp["ln_g"]),
        "lbb": bcast(inp["ln_b"]),
    }
    in_maps = []
    for core in range(_NCORES):
        b, h = divmod(core, 2)
        seq = np.asarray(inp["input_seq"][b]).astype(np.int64)
        seq = np.roll(seq, -1024 * h)
        uniq, inv = np.unique(seq, return_inverse=True)
        emb_c = np.zeros((_S, _D), f32)
        emb_c[:len(uniq)] = emb_full[uniq]
        emb_c[:len(uniq)] *= f32(_SQRT_D)
        m = dict(shared)
        m["emb"] = emb_c.astype(ml_dtypes.bfloat16)
        m["idx"] = np.ascontiguousarray(inv.astype(np.int32).reshape(_NT, _P).T)
        m["pos"] = np.ascontiguousarray(np.roll(pos_s, -1024 * h, axis=0))
        in_maps.append(m)
    return in_maps


def kernel(**inputs):
    from concourse.bass_utils import run_bass_kernel_spmd

    inp = {k: np.asarray(v) for k, v in inputs.items()}
    in_maps = _make_in_maps(inp)
    flags = (bool(np.all(np.asarray(inp["bk"]) == 0)),
             bool(np.all(np.asarray(inp["bv"]) == 0)),
             bool(np.all(np.asarray(inp["b2"]) == 0)),
             bool(np.all(np.asarray(inp["ln_g"]) == 1)),
             bool(np.all(np.asarray(inp["ln_b"]) == 0)))
    nc = _get_nc(flags)
    res = run_bass_kernel_spmd(nc, in_maps, core_ids=list(range(_NCORES)))
    out = np.empty((_B, _S, _D), np.float32)
    for core in range(_NCORES):
        b, h = divmod(core, 2)
        out[b, h * 1024:(h + 1) * 1024, :] = res.results[core]["out"]
    return out


if __name__ == "__main__":
    import sys
    if "--build" in sys.argv:
        import tempfile
        from concourse.bass_utils import compile_bass_kernel
        nc = _build_nc(True, True, True, True, True)
        d = tempfile.mkdtemp(prefix="enc_build_")
        print("compiling into", d)
        print("NEFF:", compile_bass_kernel(nc, d))


# revision 53
# speedup vs baseline: 1.0009x; 1.0009x over previous
"""Self-contained Trainium2 Bass kernel for a 1-layer transformer encoder.

Model (fp32 reference):
  x = (emb[input_seq] + pos) * sqrt(D)
  k = x@wk+bk ; q = x@wq+bq ; v = x@wv+bv
  scores[b,i,j] = sum_d k[b,i,d]*q[b,j,d] / sqrt(D)
  attn = softmax(scores, axis=-1) @ v
  r = LN(x + attn) ; ff = gelu(r@w1+b1)@w2+b2 ; out = LN(r + ff)

Sharding: 8 cores; core c handles batch c//2, sequence-half c%2.  Each core
receives its batch's full sequence rolled by -1024*h so its half is local
rows 0..1023 (softmax over keys is permutation-invariant, so one SPMD
program serves both halves).  QKV for the full local sequence is computed
on-core (duplicated across the pair); no collectives.

Precision: matmuls run single-pass float32r (full-rate fp32, ~9 mantissa
bits) using a fused matrix M = wk @ (wq/sqrt(D)).T (host-precomputed; the
per-row bias terms cancel in softmax except the bk term which rides in as
a rank-1 t2 correction when bk != 0):
  u = x@M ; scores = u @ x^T (+ 1*t2)
Softmax probabilities, v, the embedding table, w2 and the gelu activations
are bf16; the attention denominator comes from the exp accumulator in f32.
Measured end-to-end rel err ~4.4e-3 vs the 2e-2 gate.

Schedule notes (tlsim-guided):
- Phase 2 is software-pipelined: per step, pT transposes+copies(i-1) are
  emitted first (DVE copies lead the DVE queue so ps_t recycling never
  gates PE), then scores(i), exps(i), attn(i-1), denominator(i).
- LN1 is deferred and batched (rows 0..5 early, 6..7 on DVE at the end):
  one Sqrt act-table load instead of per-i Ln/Exp table thrash (1283ns
  per load, and exp/ln/sqrt live in different table sets).
- scores(0)+softmax(0) are prefilled at the end of phase 1 (jb0/jb1 on
  the then-idle attn-tag PSUM banks) so the first softmax hides under
  the phase-boundary drain.
- idx DMA is first on the sync queue (it gates all embedding gathers;
  per-queue DMA completion waits round up to 8-transfer generations, so
  queue order and crowding matter).  FFN weight DMAs are WAW-gated via
  1-element marker writes so they cannot front-run the gathers on the
  shared DMA fabric (queues run ahead of real time, so emission position
  alone does not delay a dependency-free DMA).
"""

import math

import ml_dtypes
import numpy as np

_B, _S, _D, _DFF, _V = 4, 2048, 512, 2048, 50257
_P = 128
_NCORES = 8
_SQRT_D = math.sqrt(_D)
_EPS = 1e-5

_NT = _S // _P          # 16 sequence tiles
_NI = (_S // 2) // _P   # 8 row tiles per core half
_KC = _D // _P          # 4 contraction chunks over D
_FC = _DFF // _P        # 16 contraction chunks over DFF
_JB = _S // 512         # 4 key blocks of 512

_CACHE = {}


def _pos_table():
    # Mirrors reference pos_embedding in float32.
    pos = np.arange(_S, dtype=np.float32)[:, None]
    i = np.arange(_D, dtype=np.float32)[None, :]
    ang = pos / np.power(np.float32(10000.0), np.float32(2.0) * i / np.float32(_D))
    even = (np.arange(_D) % 2 == 0)[None, :]
    return np.where(even, np.sin(ang), np.cos(ang)).astype(np.float32)


def _round_f32r(a):
    # float32r keeps the top 9 mantissa bits; round-to-nearest on the low 14.
    b = np.ascontiguousarray(a, dtype=np.float32).view(np.uint32)
    b = (b + np.uint32(0x2000)) & np.uint32(0xFFFFC000)
    return b.view(np.float32)


def _build_nc(zero_bk=False, zero_bv=False, zero_b2=False, unit_g=False,
              zero_lb=False):
    import concourse.bass as bass
    import concourse.mybir as mybir
    import concourse.tile as tile
    from concourse import bacc
    from concourse.masks import make_identity

    f32 = mybir.dt.float32
    f32r = mybir.dt.float32r
    bf16 = mybir.dt.bfloat16
    i32 = mybir.dt.int32
    i16 = mybir.dt.int16
    AF = mybir.ActivationFunctionType
    OP = mybir.AluOpType
    AX = mybir.AxisListType.X

    nc = bacc.Bacc("TRN2", target_bir_lowering=False, debug=False,
                   num_devices=_NCORES)

    idx_d = nc.dram_tensor("idx", [_P, _NT], i32, kind="ExternalInput")
    # Compact per-core embedding table: host gathers the <=S unique rows this
    # core's batch touches (device still performs the data-dependent gather).
    emb_d = nc.dram_tensor("emb", [_S, _D], bf16, kind="ExternalInput")
    pos_d = nc.dram_tensor("pos", [_S, _D], f32, kind="ExternalInput")
    mhi_d = nc.dram_tensor("m_hi", [_D, _D], f32r, kind="ExternalInput")
    wv_d = nc.dram_tensor("wv", [_D, _D], f32r, kind="ExternalInput")
    w1_d = nc.dram_tensor("w1", [_D, _DFF], f32r, kind="ExternalInput")
    w2_d = nc.dram_tensor("w2", [_DFF, _D], bf16, kind="ExternalInput")
    c2c_d = nc.dram_tensor("c2c", [_P, _KC], f32r, kind="ExternalInput")
    bvb_d = nc.dram_tensor("bvb", [_P, _D], f32, kind="ExternalInput")
    b1c_d = nc.dram_tensor("b1c", [_P, _FC], f32, kind="ExternalInput")
    b2b_d = nc.dram_tensor("b2b", [_P, _D], f32, kind="ExternalInput")
    gb_d = nc.dram_tensor("gb", [_P, _D], f32, kind="ExternalInput")
    lbb_d = nc.dram_tensor("lbb", [_P, _D], f32, kind="ExternalInput")
    out_d = nc.dram_tensor("out", [_S // 2, _D], f32, kind="ExternalOutput")

    with tile.TileContext(nc) as tc:
        consts = tc.alloc_tile_pool(name="consts", bufs=1)
        id_f = consts.tile([_P, _P], f32, name="id_f")
        make_identity(nc, id_f[:])
        id_r = consts.tile([_P, _P], f32r, name="id_r")
        nc.vector.tensor_copy(out=id_r[:], in_=id_f[:])
        id_b = consts.tile([_P, _P], bf16, name="id_b")
        nc.vector.tensor_copy(out=id_b[:], in_=id_f[:])
        ones_r = consts.tile([1, _P], f32, name="ones_f")
        nc.vector.memset(ones_r[:], 1.0)
        ones_rr = consts.tile([1, _P], f32r, name="ones_rr")
        nc.vector.tensor_copy(out=ones_rr[:], in_=ones_r[:])
        eps_t = consts.tile([_P, 1], f32, name="eps_t")
        nc.vector.memset(eps_t[:], _EPS)
        # Dummy 1-element Exp: hoists the exp act-table load (1283ns) to the
        # idle ACT engine at kernel start, off the first softmax's path.
        warm_e = consts.tile([1, 1], f32, name="warm_e")
        nc.scalar.activation(out=warm_e[:], in_=eps_t[0:1, 0:1], func=AF.Exp,
                             scale=1.0)
        c2c = bvb = b2b = gb = lbb = None
        if not zero_bk:
            c2c = consts.tile([_P, _KC], f32r, name="c2c")
            nc.scalar.dma_start(out=c2c[:], in_=c2c_d[:, :])
        if not zero_bv:
            bvb = consts.tile([_P, _D], f32, name="bvb")
            nc.scalar.dma_start(out=bvb[:], in_=bvb_d[:, :])
        b1c = consts.tile([_P, _FC], f32, name="b1c")
        if not zero_b2:
            b2b = consts.tile([_P, _D], f32, name="b2b")
            nc.scalar.dma_start(out=b2b[:], in_=b2b_d[:, :])
        if not (unit_g and zero_lb):
            gb = consts.tile([_P, _D], f32, name="gb")
            nc.scalar.dma_start(out=gb[:], in_=gb_d[:, :])
            lbb = consts.tile([_P, _D], f32, name="lbb")
            nc.scalar.dma_start(out=lbb[:], in_=lbb_d[:, :])

        xhalf = tc.alloc_tile_pool(name="xhalf", bufs=1)
        x_sb = xhalf.tile([_P, _NI, _D], f32r, name="x_sb")

        acts = tc.alloc_tile_pool(name="acts", bufs=1)
        xT = acts.tile([_P, _KC, _S], f32r, name="xT")
        uT = acts.tile([_P, _KC, _S // 2], f32r, name="uT")
        v_sb = acts.tile([_P, _NT, _D], bf16, name="v_sb")
        t2_sb = None if zero_bk else acts.tile([1, _S], f32r, name="t2_sb")

        # ---------------- Phase 1: embed, transpose, u = x@M, v ----------
        # idx FIRST on the sync queue: it gates every embedding gather, and
        # the shared DMA fabric serves transfers in issue order.
        p1t = tc.alloc_tile_pool(name="p1t", bufs=1)
        idx_sb = p1t.tile([_P, _NT], i32, name="idx_sb")
        nc.sync.dma_start(out=idx_sb[:], in_=idx_d[:, :])
        nc.sync.dma_start(out=b1c[:], in_=b1c_d[:, :])

        p1 = tc.alloc_tile_pool(name="p1", bufs=1)
        wv_sb = p1.tile([_P, _KC, _D], f32r, name="wv_sb")
        mhi_sb = p1.tile([_P, _KC, _D], f32r, name="mhi_sb")

        for c in range(_KC):
            nc.sync.dma_start(out=wv_sb[:, c, :], in_=wv_d[c * _P:(c + 1) * _P, :])

        psp = tc.alloc_tile_pool(name="psp", bufs=1, space="PSUM")

        def emit_u(ibl):
            rsl = slice(ibl * 512, (ibl + 1) * 512)
            for oc in range(_KC):
                ps_u = psp.tile([_P, 512], f32, name="ps_u", tag="mm", bufs=4)
                for c in range(_KC):
                    nc.tensor.matmul(out=ps_u[:],
                                     lhsT=mhi_sb[:, c, oc * _P:(oc + 1) * _P],
                                     rhs=xT[:, c, rsl],
                                     start=(c == 0), stop=(c == _KC - 1))
                nc.scalar.activation(out=uT[:, oc, rsl], in_=ps_u[:],
                                     func=AF.Identity, scale=1.0)

        def emit_v(t):
            ps_v = psp.tile([_P, 512], f32, name="ps_v", tag="mm", bufs=4)
            for c in range(_KC):
                nc.tensor.matmul(out=ps_v[:],
                                 lhsT=xT[:, c, t * _P:(t + 1) * _P],
                                 rhs=wv_sb[:, c, :],
                                 start=(c == 0), stop=(c == _KC - 1))
            if zero_bv:
                # t>=14 on ACT: keeps the DVE queue clear for the prefill maxes.
                if t % 2 == 0 and t < 14:
                    nc.vector.tensor_copy(out=v_sb[:, t, :], in_=ps_v[:])
                else:
                    nc.scalar.activation(out=v_sb[:, t, :], in_=ps_v[:],
                                         func=AF.Identity, scale=1.0)
            else:
                nc.vector.tensor_tensor(out=v_sb[:, t, :], in0=ps_v[:], in1=bvb[:],
                                        op=OP.add)

        pf_m4 = acts.tile([_P, _JB], f32, name="pf_m4")
        pf_ps = [None] * _JB

        def emit_pf(jb, tag, bufs):
            ps_sj = psp.tile([_P, 512], f32, name="ps_s", tag=tag, bufs=bufs)
            pf_ps[jb] = ps_sj
            jsl = slice(jb * 512, (jb + 1) * 512)
            for c in range(_KC):
                nc.tensor.matmul(out=ps_sj[:],
                                 lhsT=uT[:, c, 0:_P], rhs=xT[:, c, jsl],
                                 start=(c == 0), stop=(zero_bk and c == _KC - 1))
            if not zero_bk:
                nc.tensor.matmul(out=ps_sj[:], lhsT=ones_rr[0:1, :],
                                 rhs=t2_sb[0:1, jsl], start=False, stop=True)
            nc.vector.reduce_max(out=pf_m4[:, jb:jb + 1], in_=ps_sj[:], axis=AX)

        def emit_t2(jb):
            ps_m = psp.tile([_P, 512], f32, name="ps_m", tag="mm", bufs=4)
            jsl = slice(jb * 512, (jb + 1) * 512)
            for c in range(_KC):
                nc.tensor.matmul(out=ps_m[0:1, :], lhsT=c2c[:, c:c + 1],
                                 rhs=xT[:, c, jsl],
                                 start=(c == 0), stop=(c == _KC - 1))
            nc.vector.tensor_copy(out=t2_sb[0:1, jsl], in_=ps_m[0:1, :])

        for t in range(_NT):
            if t < _KC:
                # mhi staggered behind pos tiles on the ACT queue; first
                # consumed by emit_u(0) at t == 4, after these land.
                nc.scalar.dma_start(out=mhi_sb[:, t, :],
                                    in_=mhi_d[t * _P:(t + 1) * _P, :])
            xg = p1t.tile([_P, _D], bf16, name="xg", tag="xg", bufs=3)
            nc.gpsimd.indirect_dma_start(
                out=xg[:], out_offset=None, in_=emb_d[:, :],
                in_offset=bass.IndirectOffsetOnAxis(ap=idx_sb[:, t:t + 1], axis=0))
            pos_t = p1t.tile([_P, _D], f32, name="pos_t", tag="pos_t", bufs=3)
            nc.scalar.dma_start(out=pos_t[:], in_=pos_d[t * _P:(t + 1) * _P, :])
            if t < _NI:
                x_f = x_sb[:, t, :]
            else:
                x_f = p1t.tile([_P, _D], f32r, name="x_f", tag="x_f", bufs=3)[:]
            nc.vector.tensor_tensor(out=x_f, in0=xg[:], in1=pos_t[:], op=OP.add)
            ps_x = psp.tile([_P, _KC, _P], f32r, name="ps_x", tag="tp", bufs=2)
            for c in range(_KC):
                nc.tensor.transpose(out=ps_x[:, c, :], in_=x_f[:, c * _P:(c + 1) * _P],
                                    identity=id_r[:])
            sl = slice(t * _P, (t + 1) * _P)
            nc.scalar.activation(out=xT[:, :, sl], in_=ps_x[:, :, :],
                                 func=AF.Identity, scale=1.0)
            # v lags one tile so the ACT xT copy has a full tile period to
            # land before the v matmuls consume it.
            if t > 0:
                emit_v(t - 1)
            if t == 4:
                emit_u(0)
            if t == 8:
                emit_u(1)
            if t in (10, 12) and zero_bk:
                # prefill scores jb0/jb1 on the phase-1-idle attn-tag banks
                # (keys 0..1023 are transposed by t == 8)
                emit_pf(t // 2 - 5, "attn", 2)
        if not zero_bk:
            for jb in range(_JB):
                emit_t2(jb)

        # Prefill scores for jb2/jb3 (jb0/jb1 were emitted mid-loop on the
        # then-idle attn-tag banks); softmax(0) latency hides under v(15),
        # the phase-boundary drain, and scores(1).
        for jb in ((2, 3) if zero_bk else (0, 1, 2, 3)):
            emit_pf(jb, "mm", 4)
        pf_mneg = acts.tile([_P, 1], f32, name="pf_mneg")
        nc.vector.reduce_max(out=pf_mneg[:], in_=pf_m4[:, :], axis=AX, negate=True)
        emit_v(_NT - 1)
        pf_p = acts.tile([_P, _S], bf16, name="pf_p")
        pf_s4 = acts.tile([_P, _JB], f32, name="pf_s4")
        for jb in range(_JB):
            nc.scalar.activation(out=pf_p[:, jb * 512:(jb + 1) * 512],
                                 in_=pf_ps[jb][:], func=AF.Exp,
                                 bias=pf_mneg[:, 0:1], scale=1.0,
                                 accum_out=pf_s4[:, jb:jb + 1])
        pf_ssum = acts.tile([_P, 1], f32, name="pf_ssum")
        nc.vector.reduce_sum(out=pf_ssum[:], in_=pf_s4[:, :], axis=AX)
        pf_rinv = acts.tile([_P, 1], f32, name="pf_rinv")
        nc.vector.reciprocal(out=pf_rinv[:], in_=pf_ssum[:])

        p1.release()
        p1t.release()

        # ---------------- Phase 2: attention (pipelined) + batched LN1 ----
        rpool = tc.alloc_tile_pool(name="rpool", bufs=1, side="right")
        r_sb = rpool.tile([_P, _NI, _D], f32r, name="r_sb")
        mu_all = rpool.tile([_P, _NI], f32, name="mu_all")
        var_all = rpool.tile([_P, _NI], f32, name="var_all")
        std_all = rpool.tile([_P, _NI], f32, name="std_all")
        rstd_all = rpool.tile([_P, _NI], f32, name="rstd_all")
        # FFN weights: tiles here, DMAs issued mid-phase-2 from the scalar
        # queue so the transfers ride the then-idle HWDGE path instead of
        # clogging the shared DMA fabric in front of the embedding gathers.
        w1a = rpool.tile([_P, _KC, _DFF // 2], f32r, name="w1a")
        w1b = rpool.tile([_P, _KC, _DFF // 2], f32r, name="w1b")
        w2a = rpool.tile([_P, _FC // 2, _D], bf16, name="w2a")
        w2b = rpool.tile([_P, _FC // 2, _D], bf16, name="w2b")
        rT = rpool.tile([_P, _KC, _S // 2], f32r, name="rT")

        zpool = tc.alloc_tile_pool(name="zpool", bufs=1)
        z_sb = zpool.tile([_P, _NI, _D], f32, name="z_sb")

        p2 = tc.alloc_tile_pool(name="p2", bufs=1)

        def emit_ln1_batch(lo, hi, aeng=None):
            # One Sqrt over the batched variances (table loaded once), then
            # DVE reciprocal + per-row normalize.
            nc.scalar.activation(out=std_all[:, lo:hi], in_=var_all[:, lo:hi],
                                 func=AF.Sqrt, bias=eps_t[:, 0:1], scale=1.0)
            nc.vector.reciprocal(out=rstd_all[:, lo:hi], in_=std_all[:, lo:hi])
            for i in range(lo, hi):
                if unit_g and zero_lb:
                    (aeng or nc.gpsimd).tensor_scalar(out=r_sb[:, i, :], in0=z_sb[:, i, :],
                                            scalar1=mu_all[:, i:i + 1],
                                            scalar2=rstd_all[:, i:i + 1],
                                            op0=OP.subtract, op1=OP.mult)
                else:
                    t1 = p2.tile([_P, _D], f32, name="t1", tag="t1", bufs=2)
                    nc.vector.tensor_scalar(out=t1[:], in0=z_sb[:, i, :],
                                            scalar1=mu_all[:, i:i + 1],
                                            scalar2=rstd_all[:, i:i + 1],
                                            op0=OP.subtract, op1=OP.mult)
                    t2t = p2.tile([_P, _D], f32, name="t2t", tag="t2t", bufs=2)
                    nc.gpsimd.tensor_tensor(out=t2t[:], in0=t1[:], in1=gb[:],
                                            op=OP.mult)
                    nc.gpsimd.tensor_tensor(out=r_sb[:, i, :], in0=t2t[:], in1=lbb[:],
                                            op=OP.add)

        pending = (0, pf_p, pf_rinv)
        for step in range(1, _NI + 1):
            # -- pT transposes + copies for step-1 FIRST: the DVE copies lead
            #    this period's DVE queue so the ps_t recycling never gates PE --
            if pending is not None:
                (i0, p_prev, rinv_prev) = pending
                pT = p2.tile([_P, _NT, _P], bf16, name="pT", tag="pT", bufs=2)
                for g in range(4):
                    ps_t = psp.tile([_P, 4, _P], bf16, name="ps_t", tag="tp", bufs=2)
                    for q in range(4):
                        jt = 4 * g + q
                        nc.tensor.transpose(out=ps_t[:, q, :],
                                            in_=p_prev[:, jt * _P:(jt + 1) * _P],
                                            identity=id_b[:])
                    nc.vector.tensor_copy(out=pT[:, 4 * g:4 * (g + 1), :],
                                          in_=ps_t[:, :, :])

            # -- scores(step) matmuls + row maxes --------------------------
            if step < _NI:
                i = step
                isl = slice(i * _P, (i + 1) * _P)
                ps_s = []
                m4 = p2.tile([_P, _JB], f32, name="m4", tag="m4", bufs=2)
                for jb in range(_JB):
                    ps_sj = psp.tile([_P, 512], f32, name="ps_s", tag="mm", bufs=4)
                    ps_s.append(ps_sj)
                    jsl = slice(jb * 512, (jb + 1) * 512)
                    for c in range(_KC):
                        nc.tensor.matmul(out=ps_sj[:],
                                         lhsT=uT[:, c, isl], rhs=xT[:, c, jsl],
                                         start=(c == 0),
                                         stop=(zero_bk and c == _KC - 1))
                    if not zero_bk:
                        nc.tensor.matmul(out=ps_sj[:], lhsT=ones_rr[0:1, :],
                                         rhs=t2_sb[0:1, jsl], start=False, stop=True)
                    nc.vector.reduce_max(out=m4[:, jb:jb + 1], in_=ps_sj[:], axis=AX)
                mneg = p2.tile([_P, 1], f32, name="mneg", tag="mneg", bufs=2)
                nc.vector.reduce_max(out=mneg[:], in_=m4[:, :], axis=AX, negate=True)

            # -- exps for this step ---------------------------------------
            if step < _NI:
                p_cur = p2.tile([_P, _S], bf16, name="p_sb", tag="p_sb", bufs=2)
                s4 = p2.tile([_P, _JB], f32, name="s4", tag="s4", bufs=2)
                for jb in range(_JB):
                    nc.scalar.activation(out=p_cur[:, jb * 512:(jb + 1) * 512],
                                         in_=ps_s[jb][:], func=AF.Exp,
                                         bias=mneg[:, 0:1], scale=1.0,
                                         accum_out=s4[:, jb:jb + 1])

            # -- early LN1 batch: rows 0..5 have stats by now; the Sqrt's
            #    one-time table load and the Pool applies overlap the rest of
            #    the i-loop so r is ready the moment phase 3 starts ----------
            if step == _NI - 1:
                emit_ln1_batch(0, _NI - 2)

            # -- attention matmuls + residual/stats for step-1 ------------
            if pending is not None:
                ps_a = psp.tile([_P, _D], f32, name="ps_a", tag="attn", bufs=2)
                for jt in range(_NT):
                    nc.tensor.matmul(out=ps_a[:], lhsT=pT[:, jt, :],
                                     rhs=v_sb[:, jt, :],
                                     start=(jt == 0), stop=(jt == _NT - 1))
                zt = p2.tile([_P, _D], f32, name="zt", tag="zt", bufs=2)
                nc.scalar.activation(out=zt[:], in_=ps_a[:], func=AF.Identity,
                                     scale=rinv_prev[:, 0:1])
                zeng = nc.vector if i0 >= _NI - 2 else nc.gpsimd
                zeng.tensor_tensor(out=z_sb[:, i0, :], in0=zt[:],
                                   in1=x_sb[:, i0, :], op=OP.add)
                stats = p2.tile([_P, 6], f32, name="stats", tag="stats", bufs=2)
                nc.vector.bn_stats(out=stats[:], in_=z_sb[:, i0, :])
                mv = p2.tile([_P, 2], f32, name="mv", tag="mv", bufs=2)
                nc.vector.bn_aggr(out=mv[:], in_=stats[:])
                nc.vector.tensor_copy(out=mu_all[:, i0:i0 + 1], in_=mv[:, 0:1])
                nc.vector.tensor_copy(out=var_all[:, i0:i0 + 1], in_=mv[:, 1:2])

            # -- denominator for this step --------------------------------
            if step < _NI:
                ssum = p2.tile([_P, 1], f32, name="ssum", tag="ssum", bufs=2)
                nc.vector.reduce_sum(out=ssum[:], in_=s4[:, :], axis=AX)
                rinv = p2.tile([_P, 1], f32, name="rinv", tag="rinv", bufs=2)
                nc.vector.reciprocal(out=rinv[:], in_=ssum[:])
                pending = (i, p_cur, rinv)
            else:
                pending = None

            # FFN weight prefetch: WAW-gate each DMA on this step's rinv (a
            # 1-element marker write) so the transfer cannot front-run the
            # phase-1 embedding gathers on the shared DMA engines.
            def gated_wdma(wt, src_ap):
                nc.gpsimd.tensor_copy(out=wt[0:1, 0:1, 0:1],
                                      in_=pending[2][0:1, 0:1])
                nc.scalar.dma_start(out=wt[:], in_=src_ap)
            if step == 1:
                gated_wdma(w1a, w1_d[:, 0:_DFF // 2].rearrange("(c p) n -> p c n", p=_P))
            elif step == 2:
                gated_wdma(w1b, w1_d[:, _DFF // 2:].rearrange("(c p) n -> p c n", p=_P))
            elif step == 4:
                gated_wdma(w2a, w2_d[0:_DFF // 2, :].rearrange("(c p) n -> p c n", p=_P))
            elif step == 6:
                gated_wdma(w2b, w2_d[_DFF // 2:, :].rearrange("(c p) n -> p c n", p=_P))
        # final two row tiles (applies on DVE: Pool is mid-drain by now)
        emit_ln1_batch(_NI - 2, _NI, aeng=nc.vector)

        p2.release()
        zpool.release()
        acts.release()
        xhalf.release()

        # ---------------- Phase 3: FFN + LN2 ----------------
        p3 = tc.alloc_tile_pool(name="p3", bufs=1)

        def emit_rt(i):
            ps_rt = psp.tile([_P, _KC, _P], f32r, name="ps_rt", tag="tp", bufs=2)
            for c in range(_KC):
                nc.tensor.transpose(out=ps_rt[:, c, :],
                                    in_=r_sb[:, i, c * _P:(c + 1) * _P],
                                    identity=id_r[:])
            nc.vector.tensor_copy(out=rT[:, :, i * _P:(i + 1) * _P], in_=ps_rt[:, :, :])

        for i in range(4):
            emit_rt(i)
        gT0 = p3.tile([_P, _FC, 512], bf16, name="gT0")
        gT1 = p3.tile([_P, _FC, 512], bf16, name="gT1")
        for ib, gT in ((0, gT0), (1, gT1)):
            if ib == 1:
                for i in range(4, _NI):
                    emit_rt(i)
            for fc in range(_FC):
                ps_h = psp.tile([_P, 512], f32, name="ps_h", tag="mm", bufs=4)
                w1h = w1a if fc < _FC // 2 else w1b
                fcl = fc if fc < _FC // 2 else fc - _FC // 2
                for c in range(_KC):
                    nc.tensor.matmul(out=ps_h[:],
                                     lhsT=w1h[:, c, fcl * _P:(fcl + 1) * _P],
                                     rhs=rT[:, c, ib * 512:(ib + 1) * 512],
                                     start=(c == 0), stop=(c == _KC - 1))
                nc.scalar.activation(out=gT[:, fc, :], in_=ps_h[:], func=AF.Gelu,
                                     bias=b1c[:, fc:fc + 1], scale=1.0)
        for i in range(_NI):
            ib, il = divmod(i, 4)
            gT = gT0 if ib == 0 else gT1
            ps_o = psp.tile([_P, _D], f32, name="ps_o", tag="attn", bufs=2)
            for fc in range(_FC):
                w2h = w2a if fc < _FC // 2 else w2b
                fcl = fc if fc < _FC // 2 else fc - _FC // 2
                nc.tensor.matmul(out=ps_o[:],
                                 lhsT=gT[:, fc, il * _P:(il + 1) * _P],
                                 rhs=w2h[:, fcl, :],
                                 start=(fc == 0), stop=(fc == _FC - 1))
            t3 = p3.tile([_P, _D], f32, name="t3", tag="t3", bufs=2)
            nc.vector.tensor_tensor(out=t3[:], in0=ps_o[:], in1=r_sb[:, i, :],
                                    op=OP.add)
            if zero_b2:
                z2 = t3
            else:
                z2 = p3.tile([_P, _D], f32, name="z2", tag="z2", bufs=2)
                nc.gpsimd.tensor_tensor(out=z2[:], in0=t3[:], in1=b2b[:], op=OP.add)
            stats2 = p3.tile([_P, 6], f32, name="stats2", tag="stats2", bufs=2)
            nc.vector.bn_stats(out=stats2[:], in_=z2[:])
            mv2 = p3.tile([_P, 2], f32, name="mv2", tag="mv2", bufs=2)
            nc.vector.bn_aggr(out=mv2[:], in_=stats2[:])
            std2 = p3.tile([_P, 1], f32, name="std2", tag="std2", bufs=2)
            nc.scalar.activation(out=std2[:], in_=mv2[:, 1:2], func=AF.Sqrt,
                                 bias=eps_t[:, 0:1], scale=1.0)
            rstd2 = p3.tile([_P, 1], f32, name="rstd2", tag="rstd2", bufs=2)
            nc.vector.reciprocal(out=rstd2[:], in_=std2[:])
            out_t = p3.tile([_P, _D], f32, name="out_t", tag="out_t", bufs=3)
            if unit_g and zero_lb:
                nc.vector.tensor_scalar(out=out_t[:], in0=z2[:], scalar1=mv2[:, 0:1],
                                        scalar2=rstd2[:, 0:1],
                                        op0=OP.subtract, op1=OP.mult)
            else:
                t4 = p3.tile([_P, _D], f32, name="t4", tag="t4", bufs=2)
                nc.vector.tensor_scalar(out=t4[:], in0=z2[:], scalar1=mv2[:, 0:1],
                                        scalar2=rstd2[:, 0:1],
                                        op0=OP.subtract, op1=OP.mult)
                t5 = p3.tile([_P, _D], f32, name="t5", tag="t5", bufs=2)
                nc.gpsimd.tensor_tensor(out=t5[:], in0=t4[:], in1=gb[:], op=OP.mult)
                nc.gpsimd.tensor_tensor(out=out_t[:], in0=t5[:], in1=lbb[:], op=OP.add)
            nc.sync.dma_start(out=out_d[i * _P:(i + 1) * _P, :], in_=out_t[:])

        psp.release()
        p3.release()
        rpool.release()
        consts.release()

    nc.compile()
    return nc


def _get_nc(flags=(False, False, False, False, False)):
    if flags not in _CACHE:
        _CACHE[flags] = _build_nc(*flags)
    return _CACHE[flags]


def _make_in_maps(inp):
    f32 = np.float32
    emb_full = np.asarray(inp["emb"])
    pos_s = _pos_table() * f32(_SQRT_D)

    wk64 = np.asarray(inp["wk"], np.float64)
    wqp64 = np.asarray(inp["wq"], np.float64) / _SQRT_D
    m64 = wk64 @ wqp64.T
    m_hi = _round_f32r(m64.astype(np.float32))
    c2 = (wqp64 @ np.asarray(inp["bk"], np.float64)).astype(f32)

    def col(bias, nchunk):
        return np.ascontiguousarray(np.asarray(bias, f32).reshape(nchunk, _P).T)

    def bcast(bias):
        return np.ascontiguousarray(np.broadcast_to(np.asarray(bias, f32), (_P, _D)))

    shared = {
        "m_hi": np.ascontiguousarray(m_hi),
        "wv": np.ascontiguousarray(inp["wv"], dtype=f32),
        "w1": np.ascontiguousarray(inp["w1"], dtype=f32),
        "w2": np.ascontiguousarray(inp["w2"], dtype=f32).astype(ml_dtypes.bfloat16),
        "c2c": col(_round_f32r(c2), _KC),
        "bvb": bcast(inp["bv"]),
        "b1c": col(inp["b1"], _FC),
        "b2b": bcast(inp["b2"]),
        "gb": bcast(inp["ln_g"]),
        "lbb": bcast(inp["ln_b"]),
    }
    in_maps = []
    for core in range(_NCORES):
        b, h = divmod(core, 2)
        seq = np.asarray(inp["input_seq"][b]).astype(np.int64)
        seq = np.roll(seq, -1024 * h)
        uniq, inv = np.unique(seq, return_inverse=True)
        emb_c = np.zeros((_S, _D), f32)
        emb_c[:len(uniq)] = emb_full[uniq]
        emb_c[:len(uniq)] *= f32(_SQRT_D)
        m = dict(shared)
        m["emb"] = emb_c.astype(ml_dtypes.bfloat16)
        m["idx"] = np.ascontiguousarray(inv.astype(np.int32).reshape(_NT, _P).T)
        m["pos"] = np.ascontiguousarray(np.roll(pos_s, -1024 * h, axis=0))
        in_maps.append(m)
    return in_maps


def kernel(**inputs):
    from concourse.bass_utils import run_bass_kernel_spmd

    inp = {k: np.asarray(v) for k, v in inputs.items()}
    in_maps = _make_in_maps(inp)
    flags = (bool(np.all(np.asarray(inp["bk"]) == 0)),
             bool(np.all(np.asarray(inp["bv"]) == 0)),
             bool(np.all(np.asarray(inp["b2"]) == 0)),
             bool(np.all(np.asarray(inp["ln_g"]) == 1)),
             bool(np.all(np.asarray(inp["ln_b"]) == 0)))
    nc = _get_nc(flags)
    res = run_bass_kernel_spmd(nc, in_maps, core_ids=list(range(_NCORES)))
    out = np.empty((_B, _S, _D), np.float32)
    for core in range(_NCORES):
        b, h = divmod(core, 2)
        out[b, h * 1024:(h + 1) * 1024, :] = res.results[core]["out"]
    return out


if __name__ == "__main__":
    import sys
    if "--build" in sys.argv:
        import tempfile
        from concourse.bass_utils import compile_bass_kernel
        nc = _build_nc(True, True, True, True, True)
        d = tempfile.mkdtemp(prefix="enc_build_")
        print("compiling into", d)
        print("NEFF:", compile_bass_kernel(nc, d))
